# revision 4
# baseline (speedup 1.0000x reference)
"""Trainium2 Bass kernel v2 for nn_MILoss (Parzen-window mutual-information loss).

Contract: kernel(**inputs) takes the FULL inputs (fix_img [2,1,64,128,128] f32,
reg_img same, rand_index [2,200000] int64) and returns the FULL output (scalar
f32), sharding internally across 8 NeuronCores.

v2 strategy (per core): core g handles sample b = g//4 and a 50k block of the
200k sampled indices; (x,y) pairs are gathered host-side into xy [128, 2, F].
Device computes bin index r/c and the four 2x2-patch quadrant weights
w_ab = relu(p_a*q_b - e^-0.25) (separable Gaussians, bf16), then scatters via
one-hot matmuls accumulating a [40, 160] PSUM histogram (4 shifted blocks).

Key layout choice: the one-hot blocks are BIN-MAJOR [128, bins, CH] so every
broadcast operand (r, c, w per sample) has a step-1 inner axis -> all DVE
tensor_tensor ops run in 2x packed mode and no ACT widening copies are needed.
All PE-facing data is bf16 (fp16 hits a ~4x slower PE path).

Partial histograms are AllReduce'd across the 8 cores; every core computes the
final MI redundantly: MI = (T1 - T2x - T2y)/S + ln S.
"""

import math
from contextlib import ExitStack

import numpy as np

import concourse.bass as bass
import concourse.bacc as bacc
import concourse.mybir as mybir
import concourse.tile as tile
from concourse.bass_utils import run_bass_kernel_spmd

AF = mybir.ActivationFunctionType
ALU = mybir.AluOpType
DT = mybir.dt

NB = 40
CREL = math.exp(-0.25)
SQ2 = 0.7071067811865476

N_VOX = 1 * 64 * 128 * 128  # 1048576
N_IDX = 200000
N_CORES = 8
CORES_PER_B = 4
N_REAL = N_IDX // CORES_PER_B  # 50000 per core
FV = 400  # 128*400 = 51200 sample slots (1200 padded with 9.0 -> bin 359, no match)
CH = 50  # chunk columns (even, for DVE 2x packing)
T = FV // CH  # 8 chunks


def build_mi_kernel(n_cores=N_CORES):
    nc = bacc.Bacc(None)
    xy_d = nc.declare_dram_parameter("xy", [128, 2, FV], DT.float32, isOutput=False)
    out_d = nc.declare_dram_parameter("out", [1, 1], DT.float32, isOutput=True)

    with tile.TileContext(nc) as tc, ExitStack() as ctx:
        pools = {}

        def P(name, bufs, space="SBUF"):
            if name not in pools:
                pools[name] = ctx.enter_context(
                    tc.tile_pool(name=name, bufs=bufs, space=space)
                )
            return pools[name]

        cst = P("cst", 1)
        # iota_bm[p, j, k] = j  (bin-major iota, constant across chunks)
        iota_i = cst.tile([128, NB, CH], DT.int32, tag="iota_i")
        nc.gpsimd.iota(iota_i[:], pattern=[[1, NB], [0, CH]], base=0, channel_multiplier=0)
        iota_bm = cst.tile([128, NB, CH], DT.bfloat16, tag="iota_bm")
        nc.vector.tensor_copy(iota_bm[:], iota_i[:])
        nsq2 = cst.tile([128, 1], DT.float32, tag="nsq2")
        nc.vector.memset(nsq2[:], -SQ2)

        psum = P("psum", 1, space="PSUM")
        # Two histogram accumulators on distinct PE column-groups so LDWEIGHTS/
        # drain of one group overlaps the matmul of the other (col-tiling).
        hist2_ps = psum.tile([128, 4 * NB], DT.float32, tag="hist2")
        hist_a = hist2_ps[0:NB, :]
        hist_b = hist2_ps[64 : 64 + NB, :]

        sm = P("small", 1)
        big = P("big", 4)

        # ---- small stage, in two halves so the big stage starts earlier ----
        xy_sb = sm.tile([128, 2, FV], DT.float32, tag="xy")
        rbf = sm.tile([128, 2, FV], DT.bfloat16, tag="rbf")
        e = sm.tile([128, 2, 2, FV], DT.bfloat16, tag="e")
        w = sm.tile([128, 2, 2, FV], DT.bfloat16, tag="w")
        HF = FV // 2
        for h in range(2):
            s = slice(h * HF, (h + 1) * HF)
            nc.sync.dma_start(xy_sb[:, :, s], xy_d[:, :, s])
            # u = 40*t - 0.5
            u = sm.tile([128, 2, HF], DT.float32, tag=f"u{h}")
            nc.vector.tensor_scalar(u[:], xy_sb[:, :, s], 40.0, -0.5, ALU.mult, ALU.add)
            # floor via cast + correction (robust to trunc or round-nearest cast)
            ri = sm.tile([128, 2, HF], DT.int32, tag=f"ri{h}")
            nc.vector.tensor_copy(ri[:], u[:])
            rf0 = sm.tile([128, 2, HF], DT.float32, tag=f"rf0{h}")
            nc.vector.tensor_copy(rf0[:], ri[:])
            g = sm.tile([128, 2, HF], DT.float32, tag=f"g{h}")
            nc.vector.tensor_tensor(g[:], rf0[:], u[:], ALU.is_gt)
            rf1 = sm.tile([128, 2, HF], DT.float32, tag=f"rf1{h}")
            nc.vector.tensor_sub(rf1[:], rf0[:], g[:])
            rf = sm.tile([128, 2, HF], DT.float32, tag=f"rf{h}")
            nc.vector.tensor_scalar_max(rf[:], rf1[:], 0.0)
            nc.vector.tensor_copy(rbf[:, :, s], rf[:])
            z = sm.tile([128, 2, HF], DT.float32, tag=f"z{h}")
            nc.vector.tensor_sub(z[:], u[:], rf[:])
            # p0 = exp(-z^2/2), p1 = exp(-(z-1)^2/2); x-half -> p, y-half -> q
            sq0 = sm.tile([128, 2, HF], DT.float32, tag=f"sq0{h}")
            nc.scalar.activation(sq0[:], z[:], AF.Square, scale=SQ2)
            sq1 = sm.tile([128, 2, HF], DT.float32, tag=f"sq1{h}")
            nc.scalar.activation(sq1[:], z[:], AF.Square, scale=SQ2, bias=nsq2[:])
            nc.scalar.activation(e[:, 0, :, s], sq0[:], AF.Exp, scale=-1.0)
            nc.scalar.activation(e[:, 1, :, s], sq1[:], AF.Exp, scale=-1.0)
            # w_ab = relu(p_a*q_b - CREL), one op for all 4 quadrants
            w_raw = sm.tile([128, 2, 2, HF], DT.bfloat16, tag=f"w_raw{h}")
            nc.vector.tensor_tensor(
                w_raw[:],
                e[:, :, 0, s].unsqueeze(2).broadcast_to([128, 2, 2, HF]),
                e[:, :, 1, s].unsqueeze(1).broadcast_to([128, 2, 2, HF]),
                ALU.mult,
            )
            nc.vector.tensor_scalar(
                w[:, :, :, s], w_raw[:], CREL, 0.0, ALU.subtract, ALU.max
            )

        # ---- big stage: bin-major one-hot blocks + matmul scatter ----
        mm_i = 0
        for t in range(T):
            k0 = t * CH
            # AC[p, s, j, k]: s=0 -> (iota == r), s=1 -> (iota == c)
            AC = big.tile([128, 2, NB, CH], DT.bfloat16, tag="AC")
            nc.vector.tensor_tensor(
                AC[:],
                iota_bm[:].unsqueeze(1).broadcast_to([128, 2, NB, CH]),
                rbf[:, :, k0 : k0 + CH].unsqueeze(2).broadcast_to([128, 2, NB, CH]),
                ALU.is_equal,
            )
            # R[p, a, b, j, k] = C0[p, j, k] * w[p, a, b, k]  (4 quadrant blocks)
            R = big.tile([128, 2, 2, NB, CH], DT.bfloat16, tag="R")
            nc.vector.tensor_tensor(
                R[:],
                AC[:, 1, :, :]
                .unsqueeze(1)
                .unsqueeze(1)
                .broadcast_to([128, 2, 2, NB, CH]),
                w[:, :, :, k0 : k0 + CH]
                .unsqueeze(3)
                .broadcast_to([128, 2, 2, NB, CH]),
                ALU.mult,
            )
            for k in range(CH):
                par = (mm_i % 2 == 1)
                nc.tensor.matmul(
                    hist_b if par else hist_a,
                    lhsT=AC[:, 0, :, k],
                    rhs=R[:, :, :, :, k],
                    start=(mm_i < 2),
                    stop=(mm_i >= T * CH - 2),
                    tile_position=(0, 64) if par else (0, 0),
                )
                mm_i += 1

        # ---- combine: H[i,j] = B00 + B01[:,j-1] + B10[i-1,:] + B11[i-1,j-1] ----
        fin = P("fin", 1)
        # first merge the two column-group accumulators: hist_b sits on PSUM
        # partitions 64-103 -> DVE copy to SBUF in place, then DMA shifts it
        # down to partitions 0-39 (DMA cannot read PSUM directly)
        hb_full = fin.tile([128, 4 * NB], DT.float32, tag="hb_full")
        nc.vector.tensor_copy(hb_full[64 : 64 + NB, :], hist_b)
        hb_sb = fin.tile([NB, 4 * NB], DT.float32, tag="hb_sb")
        nc.sync.dma_start(hb_sb[:], hb_full[64 : 64 + NB, :])
        hist_ps = fin.tile([NB, 4 * NB], DT.float32, tag="hist_m")
        nc.vector.tensor_add(hist_ps[:], hist_a, hb_sb[:])
        TA = fin.tile([NB, NB], DT.float32, tag="TA")
        nc.vector.tensor_copy(TA[:], hist_ps[:, 0:NB])
        nc.vector.tensor_add(TA[:, 1:NB], TA[:, 1:NB], hist_ps[:, NB : 2 * NB - 1])
        TB = fin.tile([NB, NB], DT.float32, tag="TB")
        nc.vector.tensor_copy(TB[:], hist_ps[:, 2 * NB : 3 * NB])
        nc.vector.tensor_add(TB[:, 1:NB], TB[:, 1:NB], hist_ps[:, 3 * NB : 4 * NB - 1])
        TBs = fin.tile([NB, NB], DT.float32, tag="TBs")
        nc.vector.memset(TBs[0:1, :], 0.0)
        nc.sync.dma_start(TBs[1:NB, :], TB[0 : NB - 1, :])
        H = fin.tile([NB, NB], DT.float32, tag="H")
        nc.vector.tensor_add(H[:], TA[:], TBs[:])

        # ---- group AllReduce: cores 0-3 reduce H of sample 0, 4-7 sample 1 ----
        dram = P("dram", 1, space="DRAM")
        cin = dram.tile([NB, NB], DT.float32, tag="cin")
        cout = dram.tile([NB, NB], DT.float32, tag="cout")
        nc.sync.dma_start(cin[:, :], H[:])
        half = n_cores // 2
        nc.gpsimd.collective_compute(
            "AllReduce",
            ALU.add,
            replica_groups=[list(range(half)), list(range(half, n_cores))],
            ins=[cin[:].opt()],
            outs=[cout[:].opt()],
        )

        # ---- local MI for this core's sample: MI = (T1 - T2x - T2y)/S + ln S ----
        ones_f = cst.tile([NB, 1], DT.float32, tag="ones")
        nc.vector.memset(ones_f[:], 1.0)
        red_ps = psum.tile([1, NB], DT.float32, tag="red")
        tsum_ps = psum.tile([1, 3], DT.float32, tag="tsum")
        Hf = fin.tile([NB, NB], DT.float32, tag="Hf")
        nc.sync.dma_start(Hf[:], cout[:, :])
        Hp = fin.tile([NB, NB], DT.float32, tag="Hp")
        nc.vector.tensor_scalar_max(Hp[:], Hf[:], 1e-30)
        L = fin.tile([NB, NB], DT.float32, tag="L")
        nc.scalar.activation(L[:], Hp[:], AF.Ln)
        HL = fin.tile([NB, NB], DT.float32, tag="HL")
        nc.vector.tensor_mul(HL[:], Hp[:], L[:])
        colv = fin.tile([NB, 3], DT.float32, tag="colv")
        nc.vector.tensor_reduce(
            colv[:, 0:1], HL[:], op=ALU.add, axis=mybir.AxisListType.X
        )
        nc.vector.tensor_reduce(
            colv[:, 2:3], Hp[:], op=ALU.add, axis=mybir.AxisListType.X
        )
        Lx = fin.tile([NB, 1], DT.float32, tag="Lx")
        nc.scalar.activation(Lx[:], colv[:, 2:3], AF.Ln)
        nc.vector.tensor_mul(colv[:, 1:2], colv[:, 2:3], Lx[:])
        nc.tensor.matmul(red_ps[:], lhsT=ones_f[:], rhs=Hp[:], start=True, stop=True)
        hy = fin.tile([1, NB], DT.float32, tag="hy")
        nc.vector.tensor_copy(hy[:], red_ps[:])
        Ly = fin.tile([1, NB], DT.float32, tag="Ly")
        nc.scalar.activation(Ly[:], hy[:], AF.Ln)
        HLy = fin.tile([1, NB], DT.float32, tag="HLy")
        nc.vector.tensor_mul(HLy[:], hy[:], Ly[:])
        t2y = fin.tile([1, 1], DT.float32, tag="t2y")
        nc.vector.tensor_reduce(t2y[:], HLy[:], op=ALU.add, axis=mybir.AxisListType.X)
        nc.tensor.matmul(tsum_ps[:], lhsT=ones_f[:], rhs=colv[:], start=True, stop=True)
        tv = fin.tile([1, 3], DT.float32, tag="tv")
        nc.vector.tensor_copy(tv[:], tsum_ps[:])
        num = fin.tile([1, 1], DT.float32, tag="num")
        nc.vector.tensor_sub(num[:], tv[:, 0:1], tv[:, 1:2])
        nc.vector.tensor_sub(num[:], num[:], t2y[:])
        lS = fin.tile([1, 1], DT.float32, tag="lS")
        nc.scalar.activation(lS[:], tv[:, 2:3], AF.Ln)
        iS = fin.tile([1, 1], DT.float32, tag="iS")
        nc.vector.reciprocal(iS[:], tv[:, 2:3])
        mi = fin.tile([1, 1], DT.float32, tag="mi")
        nc.vector.tensor_mul(mi[:], num[:], iS[:])
        mi_loc = fin.tile([1, 1], DT.float32, tag="mi_loc")
        nc.vector.tensor_add(mi_loc[:], mi[:], lS[:])

        # ---- pair AllReduce (core g <-> g+4) sums the two samples' MIs ----
        cin2 = dram.tile([1, 1], DT.float32, tag="cin2")
        cout2 = dram.tile([1, 1], DT.float32, tag="cout2")
        nc.sync.dma_start(cin2[:, :], mi_loc[:])
        nc.gpsimd.collective_compute(
            "AllReduce",
            ALU.add,
            replica_groups=[[g, g + half] for g in range(half)],
            ins=[cin2[:].opt()],
            outs=[cout2[:].opt()],
        )
        loss = fin.tile([1, 1], DT.float32, tag="loss")
        nc.sync.dma_start(loss[:], cout2[:, :])
        nc.vector.tensor_scalar_mul(loss[:], loss[:], -0.5)
        nc.sync.dma_start(out_d[:, :], loss[:])

    nc.finalize()
    return nc


def make_in_maps(fix_img, reg_img, rand_index):
    xf = np.asarray(fix_img, np.float32).reshape(2, -1)
    yf = np.asarray(reg_img, np.float32).reshape(2, -1)
    ridx = np.asarray(rand_index)
    in_maps = []
    pad = 128 * FV - N_REAL
    for g in range(N_CORES):
        b, q = g // CORES_PER_B, g % CORES_PER_B
        ids = ridx[b, q * N_REAL : (q + 1) * N_REAL]
        xv = np.concatenate([xf[b][ids], np.full(pad, 9.0, np.float32)])
        yv = np.concatenate([yf[b][ids], np.full(pad, 9.0, np.float32)])
        xy = np.ascontiguousarray(
            np.stack([xv.reshape(128, FV), yv.reshape(128, FV)], axis=1)
        )
        in_maps.append({"xy": xy})
    return in_maps


_NC_CACHE = {}


def _get_nc():
    if "nc" not in _NC_CACHE:
        _NC_CACHE["nc"] = build_mi_kernel()
    return _NC_CACHE["nc"]


def run_on_hw(fix_img, reg_img, rand_index, trace=False):
    nc = _get_nc()
    in_maps = make_in_maps(fix_img, reg_img, rand_index)
    res = run_bass_kernel_spmd(nc, in_maps, core_ids=list(range(N_CORES)), trace=trace)
    out = np.asarray(res.results[0]["out"], np.float32)
    return np.float32(out.reshape(-1)[0]), res


def kernel(fix_img, reg_img, rand_index):
    val, _ = run_on_hw(fix_img, reg_img, rand_index, trace=False)
    return np.asarray(val, dtype=np.float32)


# revision 6
# speedup vs baseline: 1.8152x; 1.8152x over previous
"""Trainium2 Bass kernel v4 for nn_MILoss (Parzen-window mutual-information loss).

Contract: kernel(**inputs) takes the FULL inputs (fix_img [2,1,64,128,128] f32,
reg_img same, rand_index [2,200000] int64) and returns the FULL output (scalar
f32), sharding internally across 8 NeuronCores.

Per core: core g handles sample b = g//4 and a 50k block of the 200k sampled
indices; (x,y) pairs are gathered host-side into xy [128, 2, F]. The device
computes bin indices r/c and the four 2x2-patch quadrant weights
w_ab = relu(p_a*q_b - e^-0.25) (separable Gaussians, bf16), then scatters them
via one-hot matmuls into a [40, 160] PSUM histogram (4 shifted quadrant
blocks), ping-ponging between two PE column-groups so weight loads overlap
matmuls. One-hot blocks are BIN-MAJOR [128, bins, CH] so every broadcast
operand has a step-1 inner axis -> all DVE ops run in 2x packed mode with no
widening copies. Each core DMAs its raw dual-accumulator histogram out; the
host sums the 8 partials (fp64) and applies the scalar MI formula.
"""

import math
from contextlib import ExitStack

import numpy as np

import concourse.bass as bass
import concourse.bacc as bacc
import concourse.mybir as mybir
import concourse.tile as tile
from concourse.bass_utils import run_bass_kernel_spmd

AF = mybir.ActivationFunctionType
ALU = mybir.AluOpType
DT = mybir.dt

NB = 40
CREL = math.exp(-0.25)
SQ2 = 0.7071067811865476

N_IDX = 200000
N_CORES = 8
CORES_PER_B = 4
N_REAL = N_IDX // CORES_PER_B  # 50000 per core
FV = 392  # 128*392 = 50176 slots (176 padded with 9.0 -> bin 359, never matches)
CH = 56  # chunk columns (even, for DVE 2x packing)
T = FV // CH  # 7 chunks


def build_mi_kernel(n_cores=N_CORES):
    nc = bacc.Bacc(None)
    xy_d = nc.declare_dram_parameter("xy", [128, 2, FV], DT.float32, isOutput=False)
    out_d = nc.declare_dram_parameter("out", [128, 4 * NB], DT.float32, isOutput=True)

    with tile.TileContext(nc) as tc, ExitStack() as ctx:
        pools = {}

        def P(name, bufs, space="SBUF"):
            if name not in pools:
                pools[name] = ctx.enter_context(
                    tc.tile_pool(name=name, bufs=bufs, space=space)
                )
            return pools[name]

        cst = P("cst", 1)
        # iota_bm[p, j, k] = j  (bin-major iota, constant across chunks)
        iota_i = cst.tile([128, NB, CH], DT.int32, tag="iota_i")
        nc.gpsimd.iota(iota_i[:], pattern=[[1, NB], [0, CH]], base=0, channel_multiplier=0)
        iota_bm = cst.tile([128, NB, CH], DT.bfloat16, tag="iota_bm")
        nc.vector.tensor_copy(iota_bm[:], iota_i[:])
        # biases for Square(SQ2*z' +- SQ2/2): z' = u2 - r with u2 = 40t - 1,
        # so z = z' + 0.5 and the +-0.5 shift folds into the activation bias
        bp = cst.tile([128, 1], DT.float32, tag="bp")
        nc.vector.memset(bp[:], SQ2 / 2)
        bm = cst.tile([128, 1], DT.float32, tag="bm")
        nc.vector.memset(bm[:], -SQ2 / 2)

        psum = P("psum", 1, space="PSUM")
        # Two histogram accumulators on distinct PE column-groups so LDWEIGHTS/
        # drain of one group overlaps the matmul of the other (col-tiling).
        hist2_ps = psum.tile([128, 4 * NB], DT.float32, tag="hist2")
        hist_a = hist2_ps[0:NB, :]
        hist_b = hist2_ps[64 : 64 + NB, :]

        sm = P("small", 1)
        big = P("big", 4)

        # ---- small stage (whole core); the HW float->int cast rounds to
        # nearest, so round(40t - 1) = floor(40t - 0.5) = the reference's
        # bin index (clamped at 0 below) ----
        xy_sb = sm.tile([128, 2, FV], DT.float32, tag="xy")
        nc.sync.dma_start(xy_sb[:], xy_d[:])
        u2 = sm.tile([128, 2, FV], DT.float32, tag="u2")
        nc.vector.tensor_scalar(u2[:], xy_sb[:], 40.0, -1.0, ALU.mult, ALU.add)
        ri = sm.tile([128, 2, FV], DT.int32, tag="ri")
        nc.vector.tensor_copy(ri[:], u2[:])
        rf0 = sm.tile([128, 2, FV], DT.float32, tag="rf0")
        nc.vector.tensor_copy(rf0[:], ri[:])
        rf = sm.tile([128, 2, FV], DT.float32, tag="rf")
        nc.vector.tensor_scalar_max(rf[:], rf0[:], 0.0)
        rbf = sm.tile([128, 2, FV], DT.bfloat16, tag="rbf")
        nc.vector.tensor_copy(rbf[:], rf[:])
        z = sm.tile([128, 2, FV], DT.float32, tag="z")
        nc.vector.tensor_sub(z[:], u2[:], rf[:])
        # p0 = exp(-(z'+.5)^2/2), p1 = exp(-(z'-.5)^2/2); x-half -> p, y -> q
        sq0 = sm.tile([128, 2, FV], DT.float32, tag="sq0")
        nc.scalar.activation(sq0[:], z[:], AF.Square, scale=SQ2, bias=bp[:])
        sq1 = sm.tile([128, 2, FV], DT.float32, tag="sq1")
        nc.scalar.activation(sq1[:], z[:], AF.Square, scale=SQ2, bias=bm[:])
        e = sm.tile([128, 2, 2, FV], DT.bfloat16, tag="e")
        nc.scalar.activation(e[:, 0, :, :], sq0[:], AF.Exp, scale=-1.0)
        nc.scalar.activation(e[:, 1, :, :], sq1[:], AF.Exp, scale=-1.0)
        # w_ab = relu(p_a*q_b - CREL), one op for all 4 quadrants
        w_raw = sm.tile([128, 2, 2, FV], DT.bfloat16, tag="w_raw")
        nc.vector.tensor_tensor(
            w_raw[:],
            e[:, :, 0, :].unsqueeze(2).broadcast_to([128, 2, 2, FV]),
            e[:, :, 1, :].unsqueeze(1).broadcast_to([128, 2, 2, FV]),
            ALU.mult,
        )
        w = sm.tile([128, 2, 2, FV], DT.bfloat16, tag="w")
        nc.vector.tensor_scalar(w[:], w_raw[:], CREL, 0.0, ALU.subtract, ALU.max)

        # ---- big stage: bin-major one-hot blocks + ping-pong matmul scatter ----
        mm_i = 0
        for t in range(T):
            k0 = t * CH
            # AC[p, s, j, k]: s=0 -> (iota == r), s=1 -> (iota == c)
            AC = big.tile([128, 2, NB, CH], DT.bfloat16, tag="AC")
            nc.vector.tensor_tensor(
                AC[:],
                iota_bm[:].unsqueeze(1).broadcast_to([128, 2, NB, CH]),
                rbf[:, :, k0 : k0 + CH].unsqueeze(2).broadcast_to([128, 2, NB, CH]),
                ALU.is_equal,
            )
            # R[p, a, b, j, k] = C0[p, j, k] * w[p, a, b, k]  (4 quadrant blocks)
            R = big.tile([128, 2, 2, NB, CH], DT.bfloat16, tag="R")
            nc.vector.tensor_tensor(
                R[:],
                AC[:, 1, :, :]
                .unsqueeze(1)
                .unsqueeze(1)
                .broadcast_to([128, 2, 2, NB, CH]),
                w[:, :, :, k0 : k0 + CH]
                .unsqueeze(3)
                .broadcast_to([128, 2, 2, NB, CH]),
                ALU.mult,
            )
            for k in range(CH):
                par = (mm_i % 2 == 1)
                nc.tensor.matmul(
                    hist_b if par else hist_a,
                    lhsT=AC[:, 0, :, k],
                    rhs=R[:, :, :, :, k],
                    start=(mm_i < 2),
                    stop=(mm_i >= T * CH - 2),
                    tile_position=(0, 64) if par else (0, 0),
                )
                mm_i += 1

        # ---- export both raw accumulators; host combines + computes MI ----
        fin = P("fin", 1)
        hout = fin.tile([128, 4 * NB], DT.float32, tag="hout")
        nc.vector.memset(hout[:], 0.0)
        nc.vector.tensor_copy(hout[0:NB, :], hist_a)
        nc.vector.tensor_copy(hout[64 : 64 + NB, :], hist_b)
        nc.sync.dma_start(out_d[:, :], hout[:])

    nc.finalize()
    return nc


def make_in_maps(fix_img, reg_img, rand_index):
    xf = np.asarray(fix_img, np.float32).reshape(2, -1)
    yf = np.asarray(reg_img, np.float32).reshape(2, -1)
    ridx = np.asarray(rand_index)
    in_maps = []
    pad = 128 * FV - N_REAL
    for g in range(N_CORES):
        b, q = g // CORES_PER_B, g % CORES_PER_B
        ids = ridx[b, q * N_REAL : (q + 1) * N_REAL]
        xv = np.concatenate([xf[b][ids], np.full(pad, 9.0, np.float32)])
        yv = np.concatenate([yf[b][ids], np.full(pad, 9.0, np.float32)])
        xy = np.ascontiguousarray(
            np.stack([xv.reshape(128, FV), yv.reshape(128, FV)], axis=1)
        )
        in_maps.append({"xy": xy})
    return in_maps


def _mi_from_hist(hg):
    """Reference MI formula on a [40,40] histogram (fp64)."""
    pxy = (hg / hg.sum()).reshape(NB, NB)
    px = pxy.sum(axis=1, keepdims=True)
    py = pxy.sum(axis=0, keepdims=True)
    return -np.sum(pxy * np.log(pxy + 1e-9) - pxy * np.log(px * py + 1e-9))


def _combine_quadrants(raw):
    """raw [128, 160]: two accumulators (partitions 0-39 and 64-103), each
    holding blocks [B00 B01 B10 B11]; returns the combined [40,40] hist."""
    acc = raw[0:NB, :].astype(np.float64) + raw[64 : 64 + NB, :].astype(np.float64)
    TA = acc[:, 0:NB].copy()
    TA[:, 1:NB] += acc[:, NB : 2 * NB - 1]
    TB = acc[:, 2 * NB : 3 * NB].copy()
    TB[:, 1:NB] += acc[:, 3 * NB : 4 * NB - 1]
    H = TA
    H[1:NB, :] += TB[0 : NB - 1, :]
    return H


_NC_CACHE = {}


def _get_nc():
    if "nc" not in _NC_CACHE:
        _NC_CACHE["nc"] = build_mi_kernel()
    return _NC_CACHE["nc"]


def run_on_hw(fix_img, reg_img, rand_index, trace=False):
    nc = _get_nc()
    in_maps = make_in_maps(fix_img, reg_img, rand_index)
    res = run_bass_kernel_spmd(nc, in_maps, core_ids=list(range(N_CORES)), trace=trace)
    H = [np.zeros((NB, NB), np.float64), np.zeros((NB, NB), np.float64)]
    for g in range(N_CORES):
        raw = np.asarray(res.results[g]["out"], np.float32)
        H[g // CORES_PER_B] += _combine_quadrants(raw)
    loss = np.float64(_mi_from_hist(H[0]) + _mi_from_hist(H[1])) / 2.0
    return np.float32(loss), res


def kernel(fix_img, reg_img, rand_index):
    val, _ = run_on_hw(fix_img, reg_img, rand_index, trace=False)
    return np.asarray(val, dtype=np.float32)
